# revision 5
# baseline (speedup 1.0000x reference)
"""Trainium2 Bass kernel for nn_EdgeModel (gnn_message_passing).

Computes, for each edge e:
    x  = concat([src[e], dst[e], edge_attr[e], u[batch[e]]])   # [512]
    h  = softplus(x @ W1 + b1)                                 # [256]
    h  = softplus(h @ W2 + b2)                                 # [256]
    out[e] = h @ W3 + b3                                       # [128]

Strategy (8 NeuronCores, data-parallel over edges):
  - Host pads E=400000 -> 409600 and gives each core 51200 contiguous edges.
  - Per core: 25 supertiles of [128 partitions x 16 edges]. Each partition
    owns a contiguous edge range so HBM loads/stores are large contiguous
    per-partition transfers (1MB per input per supertile DMA).
  - fp32->bf16 cast happens inside the SWDGE DMA datapath on load; matmuls
    run in bf16 on the PE with fp32 PSUM accumulation; the store casts
    bf16->fp32 in the DMA datapath.
  - Activations are kept feature-major for the PE via xbar DMA transposes.
  - u[batch] enters layer 1 as a one-hot matmul against uW1 = u @ W1[384:],
    computed on-device once.
  - softplus(z) = Ln(1 + Exp(z)); both functions live in the same ACT
    table set, evaluated over 1024-edge megagroups to amortize overhead.
"""

import numpy as np

import concourse.bass as bass
import concourse.mybir as mybir
import concourse.tile as tile
from concourse import bacc
from concourse.bass_utils import run_bass_kernel_spmd
from concourse.masks import make_identity

# ---- problem constants ----
F = 128            # feature width of src / dst / edge_attr / u
D_HID = 256
D_OUT = 128
N_GRAPHS = 64
E_TOT = 400000
N_CORES = 8

# ---- tiling ----
P = 128            # SBUF partitions; each holds T contiguous edges/supertile
T_DEF = 16         # edge columns per partition per supertile (must be %8)
S_DEF = 25         # supertiles per core
MEGA = 1024        # edges per megagroup (8 transposed 128-col blocks)

BF16 = mybir.dt.bfloat16
F32 = mybir.dt.float32
I32 = mybir.dt.int32
AFT = mybir.ActivationFunctionType
ALU = mybir.AluOpType


def _edge_kernel(tc, aps, S, T):
    nc = tc.nc
    M = T // 8                      # megagroups per supertile
    src, dst, ea, bat, u, W1, b1, W2, b2, W3, b3, out = aps

    with tc.tile_pool(name="const", bufs=1) as cp:
        # ---- one-time setup: weights (bf16), biases, uW1 table, iota ----
        w1 = cp.tile([P, 4 * D_HID], BF16)          # w1[p,(k,h)] = W1[k*128+p, h]
        nc.gpsimd.dma_start(out=w1.rearrange("p (k h) -> p k h", k=4),
                            in_=W1.rearrange("(k p) h -> p k h", p=P))
        w2 = cp.tile([P, 2 * D_HID], BF16)
        nc.gpsimd.dma_start(out=w2.rearrange("p (k h) -> p k h", k=2),
                            in_=W2.rearrange("(k p) h -> p k h", p=P))
        w3 = cp.tile([P, 2 * D_OUT], BF16)
        nc.gpsimd.dma_start(out=w3.rearrange("p (k h) -> p k h", k=2),
                            in_=W3.rearrange("(k p) h -> p k h", p=P))
        b1c = cp.tile([P, 2], F32)                  # b1c[p, half] = b1[half*128+p]
        nc.sync.dma_start(out=b1c, in_=b1.rearrange("(h p) -> p h", p=P))
        b2c = cp.tile([P, 2], F32)
        nc.sync.dma_start(out=b2c, in_=b2.rearrange("(h p) -> p h", p=P))
        b3c = cp.tile([P, 1], F32)
        nc.sync.dma_start(out=b3c, in_=b3.rearrange("(h p) -> p h", p=P))

        ub = cp.tile([N_GRAPHS, F], BF16)
        nc.gpsimd.dma_start(out=ub, in_=u)
        ident = cp.tile([N_GRAPHS, N_GRAPHS], BF16)
        make_identity(nc, ident)

        with tc.tile_pool(name="setup_ps", bufs=1, space="PSUM") as sp:
            ut_ps = sp.tile([F, N_GRAPHS], BF16)    # u.T via PE transpose
            nc.tensor.transpose(ut_ps, ub, ident)
            ut = cp.tile([F, N_GRAPHS], BF16)
            nc.vector.tensor_copy(ut, ut_ps)
            uw_ps = sp.tile([N_GRAPHS, D_HID], F32)  # uW1 = u @ W1[384:512]
            nc.tensor.matmul(out=uw_ps, lhsT=ut, rhs=w1[:, 3 * D_HID:4 * D_HID],
                             start=True, stop=True)
            uw = cp.tile([N_GRAPHS, D_HID], BF16)
            nc.vector.tensor_copy(uw, uw_ps)

        iot_i = cp.tile([N_GRAPHS, 1], I32)
        nc.gpsimd.iota(iot_i, pattern=[[0, 1]], base=0, channel_multiplier=1)
        iot = cp.tile([N_GRAPHS, 1], F32)
        nc.vector.tensor_copy(iot, iot_i)

        # ---- main loop ----
        with tc.tile_pool(name="ins", bufs=2) as pin, \
             tc.tile_pool(name="outs", bufs=2) as pout, \
             tc.tile_pool(name="bseq", bufs=2) as pb, \
             tc.tile_pool(name="work", bufs=2) as pw, \
             tc.tile_pool(name="xt", bufs=2) as pxt, \
             tc.tile_pool(name="tmp", bufs=3) as ptmp, \
             tc.tile_pool(name="h", bufs=2) as ph, \
             tc.tile_pool(name="psA", bufs=2, space="PSUM") as pA, \
             tc.tile_pool(name="psB", bufs=2, space="PSUM") as pB:
            for s in range(S):
                base = s * P * T
                ins = []
                for name, dram in (("src", src), ("dst", dst), ("ea", ea)):
                    t_in = pin.tile([P, T * F], BF16, tag=name)
                    nc.gpsimd.dma_start(
                        out=t_in.rearrange("p (t f) -> p t f", t=T),
                        in_=dram[base:base + P * T].rearrange(
                            "(p t) f -> p t f", p=P))
                    ins.append(t_in)
                bseq = pb.tile([1, P * T], F32)      # batch, mega-ordered
                nc.sync.dma_start(
                    out=bseq,
                    in_=bat[base:base + P * T].rearrange("(a n) -> a n", a=1))
                o_sb = pout.tile([P, T * F], BF16)

                for m in range(M):
                    # one-hot [64, 1024] for u[batch]
                    bb = pw.tile([N_GRAPHS, MEGA], F32, tag="bb")
                    nc.gpsimd.partition_broadcast(
                        bb, bseq[0:1, m * MEGA:(m + 1) * MEGA])
                    oh = pw.tile([N_GRAPHS, MEGA], BF16, tag="oh")
                    nc.vector.tensor_scalar(oh, bb, iot, None, ALU.is_equal)

                    # inputs -> feature-major [128f, 1024e] (xbar transposes)
                    xts = []
                    for i, t_in in enumerate(ins):
                        xt = pxt.tile([P, MEGA], BF16, tag=f"xt{i}")
                        for j in range(8):
                            c = (8 * m + j) * 128
                            nc.sync.dma_start_transpose(
                                out=xt[:, j * 128:(j + 1) * 128],
                                in_=t_in[:, c:c + 128])
                        xts.append(xt)

                    # layer 1 + softplus
                    h1 = []
                    for half in range(2):
                        ps = pA.tile([P, MEGA], F32, tag="ps")  # 2 PSUM banks
                        for c in range(2):
                            csl = slice(c * 512, (c + 1) * 512)
                            for kc in range(3):
                                o0 = kc * D_HID + half * 128
                                nc.tensor.matmul(out=ps[:, csl],
                                                 lhsT=w1[:, o0:o0 + 128],
                                                 rhs=xts[kc][:, csl],
                                                 start=(kc == 0), stop=False)
                            nc.tensor.matmul(
                                out=ps[:, csl],
                                lhsT=uw[:, half * 128:(half + 1) * 128],
                                rhs=oh[:, csl], start=False, stop=True)
                        tmp = ptmp.tile([P, MEGA], F32)
                        nc.scalar.activation(tmp, ps, AFT.Exp,
                                             bias=b1c[:, half:half + 1],
                                             scale=1.0)
                        hh = ph.tile([P, MEGA], BF16, tag=f"h1{half}")
                        nc.scalar.activation(hh, tmp, AFT.Ln, bias=1.0,
                                             scale=1.0)
                        h1.append(hh)

                    # layer 2 + softplus
                    h2 = []
                    for half in range(2):
                        ps = pB.tile([P, MEGA], F32)
                        for c in range(2):
                            csl = slice(c * 512, (c + 1) * 512)
                            for kc in range(2):
                                o0 = kc * D_HID + half * 128
                                nc.tensor.matmul(out=ps[:, csl],
                                                 lhsT=w2[:, o0:o0 + 128],
                                                 rhs=h1[kc][:, csl],
                                                 start=(kc == 0),
                                                 stop=(kc == 1))
                        tmp = ptmp.tile([P, MEGA], F32)
                        nc.scalar.activation(tmp, ps, AFT.Exp,
                                             bias=b2c[:, half:half + 1],
                                             scale=1.0)
                        hh = ph.tile([P, MEGA], BF16, tag=f"h2{half}")
                        nc.scalar.activation(hh, tmp, AFT.Ln, bias=1.0,
                                             scale=1.0)
                        h2.append(hh)

                    # layer 3 (+b3 on DVE)
                    ps3 = pA.tile([P, MEGA], F32, tag="ps")
                    for c in range(2):
                        csl = slice(c * 512, (c + 1) * 512)
                        for kc in range(2):
                            nc.tensor.matmul(out=ps3[:, csl],
                                             lhsT=w3[:, kc * 128:(kc + 1) * 128],
                                             rhs=h2[kc][:, csl],
                                             start=(kc == 0), stop=(kc == 1))
                    h3 = ph.tile([P, MEGA], BF16, tag="h3")
                    nc.vector.tensor_scalar(h3, ps3, b3c[:, 0:1], None, ALU.add)

                    # back to edge-major into the output supertile
                    for j in range(8):
                        c = (8 * m + j) * 128
                        nc.scalar.dma_start_transpose(
                            out=o_sb[:, c:c + 128],
                            in_=h3[:, j * 128:(j + 1) * 128])

                # store supertile (bf16 -> f32 cast in DMA)
                nc.gpsimd.dma_start(
                    out=out[base:base + P * T].rearrange("(p t) f -> p t f", p=P),
                    in_=o_sb.rearrange("p (t f) -> p t f", t=T))


def build_program(S=S_DEF, T=T_DEF):
    """Build + compile the per-core Bass program (same NEFF on all cores)."""
    EC = S * P * T
    nc = bacc.Bacc("TRN2", target_bir_lowering=False, debug=False,
                   num_devices=N_CORES)
    src = nc.dram_tensor("src", [EC, F], F32, kind="ExternalInput").ap()
    dst = nc.dram_tensor("dst", [EC, F], F32, kind="ExternalInput").ap()
    ea = nc.dram_tensor("edge_attr", [EC, F], F32, kind="ExternalInput").ap()
    bat = nc.dram_tensor("batch_r", [EC], F32, kind="ExternalInput").ap()
    u = nc.dram_tensor("u", [N_GRAPHS, F], F32, kind="ExternalInput").ap()
    W1 = nc.dram_tensor("W1", [4 * F, D_HID], F32, kind="ExternalInput").ap()
    b1 = nc.dram_tensor("b1", [D_HID], F32, kind="ExternalInput").ap()
    W2 = nc.dram_tensor("W2", [D_HID, D_HID], F32, kind="ExternalInput").ap()
    b2 = nc.dram_tensor("b2", [D_HID], F32, kind="ExternalInput").ap()
    W3 = nc.dram_tensor("W3", [D_HID, D_OUT], F32, kind="ExternalInput").ap()
    b3 = nc.dram_tensor("b3", [D_OUT], F32, kind="ExternalInput").ap()
    out = nc.dram_tensor("out", [EC, F], F32, kind="ExternalOutput").ap()

    with tile.TileContext(nc) as tc:
        _edge_kernel(tc, (src, dst, ea, bat, u, W1, b1, W2, b2, W3, b3, out),
                     S, T)
    nc.compile()
    return nc


def make_in_maps(src, dst, edge_attr, u, batch, W1, b1, W2, b2, W3, b3,
                 S=S_DEF, T=T_DEF):
    """Pad + shard full inputs into per-core input maps."""
    M = T // 8
    EC = S * P * T
    e_pad = N_CORES * EC
    pad = e_pad - src.shape[0]
    assert pad >= 0

    def prep(a):
        a = np.ascontiguousarray(np.asarray(a, dtype=np.float32))
        if pad:
            a = np.concatenate(
                [a, np.zeros((pad,) + a.shape[1:], np.float32)], axis=0)
        return a

    srcp = prep(src)
    dstp = prep(dst)
    eap = prep(edge_attr)
    batp = np.concatenate(
        [np.asarray(batch).astype(np.float32), np.zeros(pad, np.float32)])
    shared = {
        "u": np.ascontiguousarray(np.asarray(u, np.float32)),
        "W1": np.ascontiguousarray(np.asarray(W1, np.float32)),
        "b1": np.ascontiguousarray(np.asarray(b1, np.float32)),
        "W2": np.ascontiguousarray(np.asarray(W2, np.float32)),
        "b2": np.ascontiguousarray(np.asarray(b2, np.float32)),
        "W3": np.ascontiguousarray(np.asarray(W3, np.float32)),
        "b3": np.ascontiguousarray(np.asarray(b3, np.float32)),
    }
    in_maps = []
    for c in range(N_CORES):
        sl = slice(c * EC, (c + 1) * EC)
        # batch reordered to the transposed-column order: [S,P,M,8]->[S,M,8,P]
        bc = batp[sl].reshape(S, P, M, 8).transpose(0, 2, 3, 1).reshape(-1)
        mm = {"src": srcp[sl], "dst": dstp[sl], "edge_attr": eap[sl],
              "batch_r": np.ascontiguousarray(bc)}
        mm.update(shared)
        in_maps.append(mm)
    return in_maps


_PROG = None


def kernel(src, dst, edge_attr, u, batch, W1, b1, W2, b2, W3, b3):
    global _PROG
    if _PROG is None:
        _PROG = build_program()
    in_maps = make_in_maps(src, dst, edge_attr, u, batch,
                           W1, b1, W2, b2, W3, b3)
    res = run_bass_kernel_spmd(_PROG, in_maps, core_ids=list(range(N_CORES)))
    outs = [res.results[c]["out"] for c in range(N_CORES)]
    full = np.concatenate(outs, axis=0)
    return np.ascontiguousarray(full[:E_TOT])
